# revision 1
# baseline (speedup 1.0000x reference)
"""BitLinear (BitNet b1.58) forward kernel for Trainium2, 8-way tensor-parallel.

Computes  y = act_quant(x) @ weight_quant(w).T + bias  for
  x [4, 2048, 4096] f32, w [11008, 4096] f32, bias [11008] f32.

Sharding (column-parallel, per spec hint): weight/bias sharded on
out_features across the 8 NeuronCores, x replicated; per-core outputs are
concatenated on the feature axis on host.

Math notes:
 - act_quant maps each token row to integer levels n in [-127, 127]
   (the clip to [-128, 127] never binds since |x*s| <= 127 by construction);
   weight_quant maps to ternary m in {-1, 0, 1}. Both are exactly
   representable in bf16, and the PE accumulates in fp32 (max |dot| <=
   4096*127 < 2^24), so the integer dot product is EXACT. The matmul
   therefore runs in bf16 at full PE rate; the scales (amax/127 per token,
   1/wscale global) are applied to the PSUM result, and bias is added from
   a pre-replicated [128, O] tile.
 - round-to-nearest-even (matching jnp.round) is done with the fp32
   magic-number trick: (v + 1.5*2^23) - 1.5*2^23.
 - the global weight scale needs mean(|w|) over the FULL weight tensor, so a
   tiny first launch reduces |w| per shard to [128] partials per core and the
   host combines them (8*128 adds) before the main launch.
"""

import numpy as np

import concourse.mybir as mybir
import concourse.tile as tile
from concourse import bacc
from concourse.bass_utils import run_bass_kernel_spmd

NCORES = 8
B, S, DIN, DOUT = 4, 2048, 4096, 11008
T = B * S                 # 8192 tokens
O = DOUT // NCORES        # 1376 out features per core
MAGIC = 12582912.0        # 1.5 * 2**23: fp32 round-to-nearest-even shifter
EPS = 1e-5
F32 = mybir.dt.float32
BF16 = mybir.dt.bfloat16
Copy = mybir.ActivationFunctionType.Copy
Alu = mybir.AluOpType
AxisX = mybir.AxisListType.X


def _new_nc():
    return bacc.Bacc(
        "TRN2", target_bir_lowering=False, debug=False, num_devices=NCORES
    )


def build_wsum(O=O, DIN=DIN):
    """Launch A: per-core partial sums of |w| over the shard -> wpart [128, 1]."""
    nc = _new_nc()
    w = nc.dram_tensor("w", [O, DIN], F32, kind="ExternalInput")
    out = nc.dram_tensor("wpart", [128, 1], F32, kind="ExternalOutput")
    nch = (O + 127) // 128
    with tile.TileContext(nc) as tc:
        with (
            tc.tile_pool(name="wp", bufs=2) as pool,
            tc.tile_pool(name="acc", bufs=1) as apool,
        ):
            acc = apool.tile([128, nch], F32)
            nc.vector.memset(acc[:], 0.0)
            for c in range(nch):
                p = min(128, O - c * 128)
                wt = pool.tile([128, DIN], F32, tag="wt")
                nc.sync.dma_start(wt[:p], w[c * 128 : c * 128 + p, :])
                nc.vector.tensor_reduce(
                    acc[:p, c : c + 1],
                    wt[:p],
                    axis=AxisX,
                    op=Alu.add,
                    apply_absolute_value=True,
                )
            res = apool.tile([128, 1], F32)
            nc.vector.tensor_reduce(res[:], acc[:], axis=AxisX, op=Alu.add)
            nc.sync.dma_start(out[:, :], res[:])
    nc.compile()
    return nc


def build_main(T=T, DIN=DIN, O=O, x_one_transpose=True, w_one_transpose=True):
    """Launch B: the BitLinear matmul for one core's shard.

    Inputs:  x [T, DIN] f32 (replicated), w [O, DIN] f32 shard,
             bias_rep [128, O] f32 (bias shard replicated across partitions),
             wsc [128, 1] f32 (wscale replicated), winv [128, 1] f32
             ((1/wscale)/127 replicated).
    Output:  y [T, O] f32 shard.
    """
    KT = DIN // 128           # contraction tiles
    TT = T // 128             # token tiles
    nchw = (O + 127) // 128   # weight partition chunks
    n_chunks = []
    n0 = 0
    while n0 < O:
        nn = min(512, O - n0)
        n_chunks.append((n0, nn))
        n0 += nn

    nc = _new_nc()
    x = nc.dram_tensor("x", [T, DIN], F32, kind="ExternalInput")
    w = nc.dram_tensor("w", [O, DIN], F32, kind="ExternalInput")
    bias_rep = nc.dram_tensor("bias_rep", [128, O], F32, kind="ExternalInput")
    wsc = nc.dram_tensor("wsc", [128, 1], F32, kind="ExternalInput")
    winv = nc.dram_tensor("winv", [128, 1], F32, kind="ExternalInput")
    y = nc.dram_tensor("y", [T, O], F32, kind="ExternalOutput")

    with tile.TileContext(nc) as tc:
        with (
            tc.tile_pool(name="const", bufs=1) as cpool,
            tc.tile_pool(name="wqt", bufs=1) as wqt_pool,
            tc.tile_pool(name="dram", bufs=1, space="DRAM") as dram_pool,
        ):
            wsc_sb = cpool.tile([128, 1], F32)
            nc.sync.dma_start(wsc_sb[:], wsc[:, :])
            winv_sb = cpool.tile([128, 1], F32)
            nc.sync.dma_start(winv_sb[:], winv[:, :])
            bias_sb = cpool.tile([128, O], F32)
            nc.sync.dma_start(bias_sb[:], bias_rep[:, :])

            # ---- weight quantize: m = clip(round(w * wscale), -1, 1) as bf16 ----
            wq_stage = dram_pool.tile([O, DIN], BF16)
            with tc.tile_pool(name="wprep", bufs=2) as wpool:
                for c in range(nchw):
                    p = min(128, O - c * 128)
                    wt = wpool.tile([128, DIN], F32, tag="wt")
                    nc.sync.dma_start(wt[:p], w[c * 128 : c * 128 + p, :])
                    # wt = w*wscale + MAGIC  (fp32 add rounds to nearest int)
                    nc.scalar.activation(
                        wt[:p], wt[:p], Copy, bias=MAGIC, scale=wsc_sb[:p]
                    )
                    wqb = wpool.tile([128, DIN], BF16, tag="wqb")
                    # min(wt - MAGIC, 1), then max(_, -1): ternary in bf16
                    nc.vector.tensor_scalar(
                        wqb[:p], wt[:p], -MAGIC, 1.0, Alu.add, Alu.min
                    )
                    nc.vector.tensor_scalar_max(wqb[:p], wqb[:p], -1.0)
                    nc.sync.dma_start(wq_stage[c * 128 : c * 128 + p, :], wqb[:p])

            # ---- transpose weights into the resident kxn tile [128, KT, O] ----
            wqT = wqt_pool.tile([128, KT, O], BF16)
            if w_one_transpose:
                nc.sync.dma_start_transpose(wqT[:, :, :], wq_stage[:, :])
            else:
                for k in range(KT):
                    nc.sync.dma_start_transpose(
                        wqT[:, k, :], wq_stage[:, k * 128 : (k + 1) * 128]
                    )

            # ---- main loop over token tiles ----
            with (
                tc.tile_pool(name="xin", bufs=3) as xpool,
                tc.tile_pool(name="xq", bufs=3) as xqpool,
                tc.tile_pool(name="xqt", bufs=3) as tqpool,
                tc.tile_pool(name="sc", bufs=6) as spool,
                tc.tile_pool(name="yout", bufs=4) as ypool,
                tc.tile_pool(name="ps", bufs=4, space="PSUM") as pspool,
            ):
                for i in range(TT):
                    xt = xpool.tile([128, DIN], F32, tag="xt")
                    nc.sync.dma_start(xt[:], x[i * 128 : (i + 1) * 128, :])
                    amax = spool.tile([128, 1], F32, tag="amax")
                    nc.vector.tensor_reduce(
                        amax[:], xt[:], axis=AxisX, op=Alu.max,
                        apply_absolute_value=True,
                    )
                    nc.vector.tensor_scalar_max(amax[:], amax[:], EPS)
                    st = spool.tile([128, 1], F32, tag="st")
                    nc.vector.reciprocal(st[:], amax[:])
                    nc.vector.tensor_scalar_mul(st[:], st[:], 127.0)
                    # output scale per token: amax * (1/wscale)/127
                    vec = spool.tile([128, 1], F32, tag="vec")
                    nc.vector.tensor_scalar_mul(vec[:], amax[:], winv_sb[:])
                    # xt = x*st + MAGIC; xq = xt - MAGIC cast to bf16
                    nc.scalar.activation(xt[:], xt[:], Copy, bias=MAGIC, scale=st[:])
                    xq = xqpool.tile([128, DIN], BF16, tag="xq")
                    nc.vector.tensor_scalar_add(xq[:], xt[:], -MAGIC)
                    xqT = tqpool.tile([128, KT, 128], BF16, tag="xqT")
                    if x_one_transpose:
                        nc.sync.dma_start_transpose(xqT[:, :, :], xq[:, :])
                    else:
                        for k in range(KT):
                            nc.sync.dma_start(
                                xqT[:, k, :], xq[:, k * 128 : (k + 1) * 128],
                                transpose=True,
                            )
                    for (n0, nn) in n_chunks:
                        ps = pspool.tile([128, 512], F32, tag="ps")
                        for k in range(KT):
                            nc.tensor.matmul(
                                ps[:, :nn],
                                lhsT=xqT[:, k, :],
                                rhs=wqT[:, k, n0 : n0 + nn],
                                start=(k == 0),
                                stop=(k == KT - 1),
                            )
                        yt = ypool.tile([128, 512], F32, tag="yt")
                        nc.vector.tensor_scalar_mul(yt[:, :nn], ps[:, :nn], vec[:])
                        nc.vector.tensor_add(
                            yt[:, :nn], yt[:, :nn], bias_sb[:, n0 : n0 + nn]
                        )
                        nc.sync.dma_start(
                            y[i * 128 : (i + 1) * 128, n0 : n0 + nn], yt[:, :nn]
                        )
    nc.compile()
    return nc


_NC_CACHE = {}


def _get_nc(name, builder):
    if name not in _NC_CACHE:
        _NC_CACHE[name] = builder()
    return _NC_CACHE[name]


def run(x, weight, bias, trace=False):
    """Returns (y_full, results_A, results_B)."""
    x2 = np.ascontiguousarray(x.reshape(T, DIN)).astype(np.float32, copy=False)
    weight = np.asarray(weight, dtype=np.float32)
    bias = np.asarray(bias, dtype=np.float32)
    core_ids = list(range(NCORES))

    w_shards = [
        np.ascontiguousarray(weight[c * O : (c + 1) * O]) for c in range(NCORES)
    ]

    # ---- launch A: global mean(|w|) via per-shard partials ----
    ncA = _get_nc("wsum", build_wsum)
    resA = run_bass_kernel_spmd(
        ncA, [{"w": w_shards[c]} for c in range(NCORES)], core_ids
    )
    total = np.float64(0.0)
    for c in range(NCORES):
        total += np.asarray(resA.results[c]["wpart"], dtype=np.float64).sum()
    mean_abs = np.float32(total / (DOUT * DIN))
    mean_abs = np.maximum(mean_abs, np.float32(EPS))
    wscale = np.float32(1.0) / mean_abs          # reference: 1/clip(mean|w|, EPS)
    winv = (np.float32(1.0) / wscale) / np.float32(127.0)

    # ---- launch B: the matmul ----
    ncB = _get_nc("main", build_main)
    wsc128 = np.full((128, 1), wscale, np.float32)
    winv128 = np.full((128, 1), winv, np.float32)
    in_maps = []
    for c in range(NCORES):
        b_rep = np.ascontiguousarray(
            np.broadcast_to(bias[c * O : (c + 1) * O], (128, O))
        ).astype(np.float32, copy=False)
        in_maps.append(
            {
                "x": x2,
                "w": w_shards[c],
                "bias_rep": b_rep,
                "wsc": wsc128,
                "winv": winv128,
            }
        )
    resB = run_bass_kernel_spmd(ncB, in_maps, core_ids, trace=trace)
    y_full = np.concatenate(
        [np.asarray(resB.results[c]["y"]) for c in range(NCORES)], axis=1
    ).reshape(B, S, DOUT)
    return y_full, resA, resB


def kernel(x, weight, bias):
    y, _, _ = run(x, weight, bias, trace=False)
    return y


# revision 23
# speedup vs baseline: 45.0803x; 45.0803x over previous
"""BitLinear (BitNet b1.58) forward kernel for Trainium2, 8-way tensor-parallel.

Computes  y = act_quant(x) @ weight_quant(w).T + bias  for
  x [4, 2048, 4096] f32, w [11008, 4096] f32, bias [11008] f32.

Sharding (column-parallel, per spec hint): weight/bias sharded on
out_features across the 8 NeuronCores, x replicated; per-core outputs are
concatenated on the feature axis on host.

Math notes:
 - act_quant maps each token row to integer levels n in [-127, 127]
   (the clip to [-128, 127] never binds since |x*s| <= 127 by construction);
   weight_quant maps to ternary m in {-1, 0, 1}. Both are exactly
   representable in bf16, and the PE accumulates in fp32 (max |dot| <=
   4096*127 < 2^24), so the integer dot product is EXACT. The matmul
   therefore runs in bf16 at full PE rate; the scales (amax/127 per token,
   1/wscale global) are applied to the PSUM result, and bias is added from
   a pre-replicated [128, O] tile.
 - round-to-nearest-even (matching jnp.round) is done with the fp32
   magic-number trick: (v + 1.5*2^23) - 1.5*2^23.
 - the global weight scale needs mean(|w|) over the FULL weight tensor, so a
   tiny first launch reduces |w| per shard to [128] partials per core and the
   host combines them (8*128 adds) before the main launch.
"""

import numpy as np

import concourse.mybir as mybir
import concourse.tile as tile
from concourse import bacc
from concourse.bass_utils import run_bass_kernel_spmd

NCORES = 8
B, S, DIN, DOUT = 4, 2048, 4096, 11008
T = B * S                 # 8192 tokens
O = DOUT // NCORES        # 1376 out features per core
MAGIC = 12582912.0        # 1.5 * 2**23: fp32 round-to-nearest-even shifter
EPS = 1e-5
F32 = mybir.dt.float32
BF16 = mybir.dt.bfloat16
FP8 = mybir.dt.float8e4  # ternary weights are exact in fp8e4
Copy = mybir.ActivationFunctionType.Copy
Alu = mybir.AluOpType
AxisX = mybir.AxisListType.X


def _new_nc():
    return bacc.Bacc(
        "TRN2", target_bir_lowering=False, debug=False, num_devices=NCORES
    )


def build_wsum(O=O, DIN=DIN):
    """Launch A: per-core partial sums of |w| over the shard -> wpart [128, 1].

    Takes the shard in transposed layout wT [DIN, O] (same layout launch B
    uses), reduced in [128, O] chunks.
    """
    nc = _new_nc()
    w = nc.dram_tensor("w", [DIN, O], F32, kind="ExternalInput")
    out = nc.dram_tensor("wpart", [128, 1], F32, kind="ExternalOutput")
    nch = DIN // 128
    with tile.TileContext(nc) as tc:
        with (
            tc.tile_pool(name="wp", bufs=3) as pool,
            tc.tile_pool(name="acc", bufs=1) as apool,
        ):
            acc = apool.tile([128, nch], F32)
            for c in range(nch):
                wt = pool.tile([128, O], F32, tag="wt")
                nc.sync.dma_start(wt[:], w[c * 128 : (c + 1) * 128, :])
                nc.vector.tensor_reduce(
                    acc[:, c : c + 1],
                    wt[:],
                    axis=AxisX,
                    op=Alu.add,
                    apply_absolute_value=True,
                )
            res = apool.tile([128, 1], F32)
            nc.vector.tensor_reduce(res[:], acc[:], axis=AxisX, op=Alu.add)
            nc.sync.dma_start(out[:, :], res[:])
    nc.compile()
    return nc


def build_main(T=T, DIN=DIN, O=O, x_one_transpose=True, repeat=1,
               probe_chunks=None):
    """Launch B: the BitLinear matmul for one core's shard.

    Inputs:  x [T, DIN] f32 (replicated), w [DIN, O] f32 shard (pre-transposed
             on host as part of the sharding layout),
             bias_rep [128, O] f32 (bias shard replicated across partitions),
             wsc [128, 1] f32 (wscale replicated), winv [128, 1] f32
             ((1/wscale)/127 replicated).
    Output:  y [T, O] f32 shard.
    """
    KT = DIN // 128           # contraction tiles
    TT = T // 128             # token tiles
    n_chunks = []
    n0 = 0
    while n0 < O:
        nn = min(512, O - n0)
        n_chunks.append((n0, nn))
        n0 += nn
    if probe_chunks is not None:  # timing probes only — wrong output
        n_chunks = n_chunks[:probe_chunks]

    nc = _new_nc()
    x = nc.dram_tensor("x", [T, DIN], F32, kind="ExternalInput")
    w = nc.dram_tensor("w", [DIN, O], F32, kind="ExternalInput")  # shard pre-transposed
    bias_rep = nc.dram_tensor("bias_rep", [128, O], F32, kind="ExternalInput")
    wsc = nc.dram_tensor("wsc", [128, 1], F32, kind="ExternalInput")
    winv = nc.dram_tensor("winv", [128, 1], F32, kind="ExternalInput")
    y = nc.dram_tensor("y", [T, O], F32, kind="ExternalOutput")

    with tile.TileContext(nc) as tc:
        with (
            tc.tile_pool(name="const", bufs=1) as cpool,
            tc.tile_pool(name="wqt", bufs=1) as wqt_pool,
            tc.tile_pool(name="wprep", bufs=2) as wpool,
        ):
            wsc_sb = cpool.tile([128, 1], F32)
            nc.sync.dma_start(wsc_sb[:], wsc[:, :])
            winv_sb = cpool.tile([128, 1], F32)
            nc.sync.dma_start(winv_sb[:], winv[:, :])
            bias_sb = cpool.tile([128, O], F32)
            nc.sync.dma_start(bias_sb[:], bias_rep[:, :])

            # ---- weight quantize: m = clip(round(wT * wscale), -1, 1) as bf16,
            #      one persistent [128, O] tile per contraction k-tile so the
            #      first matmuls are gated only on their own k-slice ----
            wqT = []
            for k in range(KT):
                wqk = wqt_pool.tile([128, O], FP8, tag=f"wqT{k}")
                wqT.append(wqk)
                wt = wpool.tile([128, O], F32, tag="wt")
                nc.sync.dma_start(wt[:], w[k * 128 : (k + 1) * 128, :])
                # wt = w*wscale + MAGIC  (fp32 add rounds to nearest int)
                nc.scalar.activation(wt[:], wt[:], Copy, bias=MAGIC, scale=wsc_sb[:])
                # min(wt - MAGIC, 1), then max(_, -1): ternary in bf16
                nc.vector.tensor_scalar(wqk[:], wt[:], -MAGIC, 1.0, Alu.add, Alu.min)
                nc.vector.tensor_scalar_max(wqk[:], wqk[:], -1.0)

            # ---- main loop over token tiles ----
            with (
                tc.tile_pool(name="xin", bufs=3) as xpool,
                tc.tile_pool(name="xq", bufs=3) as xqpool,
                tc.tile_pool(name="xqt", bufs=3) as tqpool,
                tc.tile_pool(name="sc", bufs=8) as spool,
                tc.tile_pool(name="yout", bufs=4) as ypool,
                tc.tile_pool(name="ps", bufs=6, space="PSUM") as pspool,
            ):

                def token_phase():
                    for i in range(TT):
                        token_tile(i)

                def token_tile(i):
                    xt = xpool.tile([128, DIN], F32, tag="xt")
                    nc.sync.dma_start(xt[:], x[i * 128 : (i + 1) * 128, :])
                    amax = spool.tile([128, 1], F32, tag="amax")
                    nc.vector.tensor_reduce(
                        amax[:], xt[:], axis=AxisX, op=Alu.max,
                        apply_absolute_value=True,
                    )
                    nc.vector.tensor_scalar_max(amax[:], amax[:], EPS)
                    st = spool.tile([128, 1], F32, tag="st")
                    nc.vector.reciprocal(st[:], amax[:])
                    nc.vector.tensor_scalar_mul(st[:], st[:], 127.0)
                    # output scale per token: amax * (1/wscale)/127
                    vec = spool.tile([128, 1], F32, tag="vec")
                    nc.vector.tensor_scalar_mul(vec[:], amax[:], winv_sb[:])
                    # xt = x*st + MAGIC; xq = xt - MAGIC cast to bf16
                    nc.scalar.activation(xt[:], xt[:], Copy, bias=MAGIC, scale=st[:])
                    xq = xqpool.tile([128, DIN], BF16, tag="xq")
                    nc.vector.tensor_scalar_add(xq[:], xt[:], -MAGIC)
                    xqT = tqpool.tile([128, KT, 128], BF16, tag="xqT")
                    if x_one_transpose:
                        nc.sync.dma_start_transpose(xqT[:, :, :], xq[:, :])
                    else:
                        for k in range(KT):
                            nc.sync.dma_start(
                                xqT[:, k, :], xq[:, k * 128 : (k + 1) * 128],
                                transpose=True,
                            )
                    for (n0, nn) in n_chunks:
                        ps = pspool.tile([128, 512], F32, tag="ps")
                        for k in range(KT):
                            nc.tensor.matmul(
                                ps[:, :nn],
                                lhsT=xqT[:, k, :],
                                rhs=wqT[k][:, n0 : n0 + nn],
                                start=(k == 0),
                                stop=(k == KT - 1),
                            )
                        yt = ypool.tile([128, 512], F32, tag="yt")
                        nc.vector.tensor_scalar_mul(yt[:, :nn], ps[:, :nn], vec[:])
                        nc.vector.tensor_add(
                            yt[:, :nn], yt[:, :nn], bias_sb[:, n0 : n0 + nn]
                        )
                        nc.sync.dma_start(
                            y[i * 128 : (i + 1) * 128, n0 : n0 + nn], yt[:, :nn]
                        )

                if repeat > 1:
                    with tc.For_i(0, repeat, 1):
                        token_phase()
                else:
                    token_phase()
    nc.compile()
    return nc


_NC_CACHE = {}


def _get_nc(name, builder):
    if name not in _NC_CACHE:
        _NC_CACHE[name] = builder()
    return _NC_CACHE[name]


def run(x, weight, bias, trace=False):
    """Returns (y_full, results_A, results_B)."""
    x2 = np.ascontiguousarray(x.reshape(T, DIN)).astype(np.float32, copy=False)
    weight = np.asarray(weight, dtype=np.float32)
    bias = np.asarray(bias, dtype=np.float32)
    core_ids = list(range(NCORES))

    # per-core shard of w, stored transposed [DIN, O] (column-parallel layout)
    w_shards = [
        np.ascontiguousarray(weight[c * O : (c + 1) * O].T) for c in range(NCORES)
    ]

    # ---- launch A: global mean(|w|) via per-shard partials ----
    ncA = _get_nc("wsum", build_wsum)
    resA = run_bass_kernel_spmd(
        ncA, [{"w": w_shards[c]} for c in range(NCORES)], core_ids
    )
    total = np.float64(0.0)
    for c in range(NCORES):
        total += np.asarray(resA.results[c]["wpart"], dtype=np.float64).sum()
    mean_abs = np.float32(total / (DOUT * DIN))
    mean_abs = np.maximum(mean_abs, np.float32(EPS))
    wscale = np.float32(1.0) / mean_abs          # reference: 1/clip(mean|w|, EPS)
    winv = (np.float32(1.0) / wscale) / np.float32(127.0)

    # ---- launch B: the matmul ----
    ncB = _get_nc("main", build_main)
    wsc128 = np.full((128, 1), wscale, np.float32)
    winv128 = np.full((128, 1), winv, np.float32)
    in_maps = []
    for c in range(NCORES):
        b_rep = np.ascontiguousarray(
            np.broadcast_to(bias[c * O : (c + 1) * O], (128, O))
        ).astype(np.float32, copy=False)
        in_maps.append(
            {
                "x": x2,
                "w": w_shards[c],
                "bias_rep": b_rep,
                "wsc": wsc128,
                "winv": winv128,
            }
        )
    resB = run_bass_kernel_spmd(ncB, in_maps, core_ids, trace=trace)
    y_full = np.concatenate(
        [np.asarray(resB.results[c]["y"]) for c in range(NCORES)], axis=1
    ).reshape(B, S, DOUT)
    return y_full, resA, resB


def kernel(x, weight, bias):
    y, _, _ = run(x, weight, bias, trace=False)
    return y
